# revision 16
# baseline (speedup 1.0000x reference)
"""Inverse 3D Haar wavelet transform (stride-2 kernel-2 conv_transpose) on 8 trn2 cores.

coeffs: [4, 64, 17, 128, 128] f32, channel dim = 8 subbands x 8 channels.
out:    [4, 8, 33, 256, 256] f32,
  out[b,c,2t+i-1, 2h+j, 2w+k] = 0.3536 * sum_s (-1)^(i*s2 + j*s1 + k*s0) x[b,s,c,t,h,w]
  (frame t'=-1 dropped).

Sharding: pure data parallel over the 8 channels c (one per core); each core
sees its [4, 8, 17, 128, 128] slice and emits [4, 33, 256, 256].

Per-core kernel: partition dim = h (128). For each (b, t-chunk):
  - one DMA loads all 8 subband tiles  [128h, 8*T*128]
  - ACT scales by 0.3536 in place
  - DVE butterfly stage 1 (contract s2 -> i-parity), stage 2 (s1 -> j)
  - GPSIMD butterfly stage 3 (s0 -> k) writes w-interleaved into frame tiles
  - one DMA stores the 2T assembled output frames (contiguous 2KB runs)
"""

import sys

sys.path.insert(0, "/opt/trn_rl_repo")

import numpy as np

import concourse.bass as bass
import concourse.bacc as bacc
import concourse.mybir as mybir
from concourse.tile import TileContext
from concourse import bass_utils

B, S, C, T_FULL, H, W = 4, 8, 8, 17, 128, 128
SCALE = 0.3536
T_CHUNK = 4  # t values per inner iteration

_cache = {}


def _build():
    nc = bacc.Bacc()
    x = nc.dram_tensor("x", [B, S, T_FULL, H, W], mybir.dt.float32, kind="ExternalInput")
    y = nc.dram_tensor("y", [B, 2 * T_FULL - 1, 2 * H, 2 * W], mybir.dt.float32,
                       kind="ExternalOutput")

    with TileContext(nc) as tc:
        with tc.tile_pool(name="xin", bufs=3) as xpool, \
             tc.tile_pool(name="uv", bufs=2) as uvpool, \
             tc.tile_pool(name="fr", bufs=3) as fpool:
            for b in range(B):
                t0 = 0
                while t0 < T_FULL:
                    T = min(T_CHUNK, T_FULL - t0)
                    FD = T * W
                    # ---- load: one DMA per t covering all 8 subbands (512 KB
                    #      each, 3D AP [h, s, w]); tile free layout = (t, s, w)
                    xall = xpool.tile([H, S * FD], mybir.dt.float32, tag="xall")
                    x3 = xall[:].rearrange("p (t s w) -> p t s w", s=S, w=W)
                    for tl in range(T):
                        src = x[b, :, t0 + tl].transpose([1, 0, 2])  # [h, s, w]
                        nc.sync.dma_start(out=x3[:, tl], in_=src)
                    # x_s view: [128h, (t, w)] with t-stride S*W
                    xs = [xall[:].rearrange("p (t s w) -> p s t w", s=S, w=W)[:, s]
                          for s in range(S)]
                    # (scale by 0.3536 is pre-applied on the host)
                    # ---- stage 1 on DVE: u[i][m] = x[m] +/- x[4+m]   (m = s1*2+s0)
                    u = {}
                    for i in range(2):
                        for m in range(4):
                            ut = uvpool.tile([H, FD], mybir.dt.float32, tag=f"u{i}{m}")
                            u3 = ut[:].rearrange("p (t w) -> p t w", w=W)
                            if i == 0:
                                nc.vector.tensor_add(u3, xs[m], xs[4 + m])
                            else:
                                nc.vector.tensor_sub(u3, xs[m], xs[4 + m])
                            u[i, m] = ut
                    # ---- stage 2 on DVE: v[i][j][s0] = u[i][s0] +/- u[i][2+s0]
                    v = {}
                    for i in range(2):
                        for j in range(2):
                            for s0 in range(2):
                                vt = uvpool.tile([H, FD], mybir.dt.float32,
                                                 tag=f"v{i}{j}{s0}")
                                if j == 0:
                                    nc.vector.tensor_add(vt[:], u[i, s0][:], u[i, 2 + s0][:])
                                else:
                                    nc.vector.tensor_sub(vt[:], u[i, s0][:], u[i, 2 + s0][:])
                                v[i, j, s0] = vt
                    # ---- stage 3 on GPSIMD: o[i][j][k] = v[ij0] +/- v[ij1],
                    #      written w-interleaved into the frame tile
                    # frame tile free layout: slot(2T) x [j(2) x w'(256)], slot = 2*t_local+i
                    # +8 pad columns: a tiny POOL memset "toucher" acquires the
                    # slot (absorbing the store-DMA WAR + release waits on POOL's
                    # clock) so the 8 real POOL ops stay within the 2-wait ISA cap
                    F = fpool.tile([H, 2 * T * 512 + 8], mybir.dt.float32, tag="F")
                    nc.gpsimd.memset(F[:, 2 * T * 512:], 0.0)
                    F3 = F[:, :2 * T * 512].rearrange("p (m r) -> p m r", r=512)  # [128, 2T, 512]
                    for i in range(2):
                        for j in range(2):
                            for k in range(2):
                                dst = F3[:, i::2, j * 256 + k:(j + 1) * 256:2]
                                in0 = v[i, j, 0][:].rearrange("p (t w) -> p t w", w=W)
                                in1 = v[i, j, 1][:].rearrange("p (t w) -> p t w", w=W)
                                if k == 0:
                                    nc.gpsimd.tensor_add(dst, in0, in1)
                                else:
                                    nc.gpsimd.tensor_sub(dst, in0, in1)
                    # ---- store: slot m -> output frame 2*t0 + m - 1 (drop t'=-1)
                    skip = 1 if t0 == 0 else 0
                    nf = 2 * T - skip
                    f0 = 2 * t0 - 1 + skip
                    dst = y[b, f0:f0 + nf].rearrange("f (p two) w -> p f (two w)", p=H)
                    # stores on the ACT HWDGE ring: don't queue behind loads
                    nc.scalar.dma_start(
                        out=dst, in_=F3[:, skip:2 * T, :])
                    t0 += T
    nc.finalize()  # runs the Bacc pass pipeline (splits >1-wait sync via event sems)
    return nc


def kernel(coeffs: np.ndarray) -> np.ndarray:
    coeffs = np.asarray(coeffs, dtype=np.float32)
    if "nc" not in _cache:
        _cache["nc"] = _build()
    nc = _cache["nc"]
    # fold the 0.3536 Haar synthesis scale into the per-core shard copy
    in_maps = [{"x": coeffs[:, c::8] * np.float32(SCALE)} for c in range(8)]
    res = bass_utils.run_bass_kernel_spmd(nc, in_maps, core_ids=list(range(8)))
    out = np.stack([res.results[c]["y"] for c in range(8)], axis=1)
    return out


# revision 18
# speedup vs baseline: 1.0056x; 1.0056x over previous
"""Inverse 3D Haar wavelet transform (stride-2 kernel-2 conv_transpose) on 8 trn2 cores.

coeffs: [4, 64, 17, 128, 128] f32, channel dim = 8 subbands x 8 channels.
out:    [4, 8, 33, 256, 256] f32,
  out[b,c,2t+i-1, 2h+j, 2w+k] = 0.3536 * sum_s (-1)^(i*s2 + j*s1 + k*s0) x[b,s,c,t,h,w]
  (frame t'=-1 dropped).

Sharding: pure data parallel over the 8 channels c (one per core); each core
sees its [4, 8, 17, 128, 128] slice and emits [4, 33, 256, 256].

Per-core kernel: partition dim = h (128). For each (b, t-chunk):
  - one DMA loads all 8 subband tiles  [128h, 8*T*128]
  - ACT scales by 0.3536 in place
  - DVE butterfly stage 1 (contract s2 -> i-parity), stage 2 (s1 -> j)
  - GPSIMD butterfly stage 3 (s0 -> k) writes w-interleaved into frame tiles
  - one DMA stores the 2T assembled output frames (contiguous 2KB runs)
"""

import sys

sys.path.insert(0, "/opt/trn_rl_repo")

import numpy as np

import concourse.bass as bass
import concourse.bacc as bacc
import concourse.mybir as mybir
from concourse.tile import TileContext
from concourse import bass_utils

B, S, C, T_FULL, H, W = 4, 8, 8, 17, 128, 128
SCALE = 0.3536
T_CHUNK = 4  # t values per inner iteration

_cache = {}


def _build():
    nc = bacc.Bacc()
    x = nc.dram_tensor("x", [B, S, T_FULL, H, W], mybir.dt.float32, kind="ExternalInput")
    y = nc.dram_tensor("y", [B, 2 * T_FULL - 1, 2 * H, 2 * W], mybir.dt.float32,
                       kind="ExternalOutput")

    with TileContext(nc) as tc:
        with tc.tile_pool(name="xin", bufs=3) as xpool, \
             tc.tile_pool(name="uv", bufs=3) as uvpool, \
             tc.tile_pool(name="fr", bufs=3) as fpool:
            for b in range(B):
                t0 = 0
                # [4,4,3,3,3] instead of [4,4,4,4,1]: avoids the tiny FD=128
                # runt chunk (per-op overhead dominated) at equal SBUF footprint
                for T in (4, 4, 3, 3, 3):
                    FD = T * W
                    # ---- load: one DMA per t covering all 8 subbands (512 KB
                    #      each, 3D AP [h, s, w]); tile free layout = (t, s, w)
                    xall = xpool.tile([H, S * FD], mybir.dt.float32, tag="xall")
                    x3 = xall[:].rearrange("p (t s w) -> p t s w", s=S, w=W)
                    for tl in range(T):
                        src = x[b, :, t0 + tl].transpose([1, 0, 2])  # [h, s, w]
                        nc.sync.dma_start(out=x3[:, tl], in_=src)
                    # x_s view: [128h, (t, w)] with t-stride S*W
                    xs = [xall[:].rearrange("p (t s w) -> p s t w", s=S, w=W)[:, s]
                          for s in range(S)]
                    # (scale by 0.3536 is pre-applied on the host)
                    # ---- stage 1 on DVE: u[i][m] = x[m] +/- x[4+m]   (m = s1*2+s0)
                    u = {}
                    for i in range(2):
                        for m in range(4):
                            ut = uvpool.tile([H, FD], mybir.dt.float32, tag=f"u{i}{m}")
                            u3 = ut[:].rearrange("p (t w) -> p t w", w=W)
                            if i == 0:
                                nc.vector.tensor_add(u3, xs[m], xs[4 + m])
                            else:
                                nc.vector.tensor_sub(u3, xs[m], xs[4 + m])
                            u[i, m] = ut
                    # ---- stage 2 on DVE: v[i][j][s0] = u[i][s0] +/- u[i][2+s0]
                    v = {}
                    for i in range(2):
                        for j in range(2):
                            for s0 in range(2):
                                vt = uvpool.tile([H, FD], mybir.dt.float32,
                                                 tag=f"v{i}{j}{s0}")
                                if j == 0:
                                    nc.vector.tensor_add(vt[:], u[i, s0][:], u[i, 2 + s0][:])
                                else:
                                    nc.vector.tensor_sub(vt[:], u[i, s0][:], u[i, 2 + s0][:])
                                v[i, j, s0] = vt
                    # ---- stage 3 on GPSIMD: o[i][j][k] = v[ij0] +/- v[ij1],
                    #      written w-interleaved into the frame tile
                    # frame tile free layout: slot(2T) x [j(2) x w'(256)], slot = 2*t_local+i
                    # +8 pad columns: a tiny POOL memset "toucher" acquires the
                    # slot (absorbing the store-DMA WAR + release waits on POOL's
                    # clock) so the 8 real POOL ops stay within the 2-wait ISA cap
                    F = fpool.tile([H, 2 * T * 512 + 8], mybir.dt.float32, tag="F")
                    nc.gpsimd.memset(F[:, 2 * T * 512:], 0.0)
                    F3 = F[:, :2 * T * 512].rearrange("p (m r) -> p m r", r=512)  # [128, 2T, 512]
                    for i in range(2):
                        for j in range(2):
                            for k in range(2):
                                dst = F3[:, i::2, j * 256 + k:(j + 1) * 256:2]
                                in0 = v[i, j, 0][:].rearrange("p (t w) -> p t w", w=W)
                                in1 = v[i, j, 1][:].rearrange("p (t w) -> p t w", w=W)
                                if k == 0:
                                    nc.gpsimd.tensor_add(dst, in0, in1)
                                else:
                                    nc.gpsimd.tensor_sub(dst, in0, in1)
                    # ---- store: slot m -> output frame 2*t0 + m - 1 (drop t'=-1)
                    skip = 1 if t0 == 0 else 0
                    nf = 2 * T - skip
                    f0 = 2 * t0 - 1 + skip
                    dst = y[b, f0:f0 + nf].rearrange("f (p two) w -> p f (two w)", p=H)
                    # stores on the ACT HWDGE ring: don't queue behind loads
                    nc.scalar.dma_start(
                        out=dst, in_=F3[:, skip:2 * T, :])
                    t0 += T
    nc.finalize()  # runs the Bacc pass pipeline (splits >1-wait sync via event sems)
    return nc


def kernel(coeffs: np.ndarray) -> np.ndarray:
    coeffs = np.asarray(coeffs, dtype=np.float32)
    if "nc" not in _cache:
        _cache["nc"] = _build()
    nc = _cache["nc"]
    # fold the 0.3536 Haar synthesis scale into the per-core shard copy
    in_maps = [{"x": coeffs[:, c::8] * np.float32(SCALE)} for c in range(8)]
    res = bass_utils.run_bass_kernel_spmd(nc, in_maps, core_ids=list(range(8)))
    out = np.stack([res.results[c]["y"] for c in range(8)], axis=1)
    return out
